# revision 10
# baseline (speedup 1.0000x reference)
"""DiffKS (differentiable Karplus-Strong) Trainium2 Bass kernel.

Self-contained: builds and runs a Bass/Tile kernel on 8 NeuronCores (SPMD,
identical program; the problem is a single voice so the recursion itself is
sequential — every core computes the full result, output read from core 0).

Algorithm (all value-dependent compute on device):
  - control upsampling (delay, coeffs) via constant one-hot frame matmuls +
    constant masks (index tables depend only on shapes -> baked host-side as
    constant input tensors);
  - excitation LPC (order 5, tiny coeffs) via 12 Neumann iterations;
  - main KS loop y[t] = x[t] - sum_j vals[t,j] y[t-z[t]-1-j] with min lag 101,
    processed in blocks of 96: per block an 8-core GPSIMD ap_gather fetches
    the 7 taps (+x as the 8th tap) for all 96 samples, DVE applies the
    coefficients, one fp32 ones-matmul reduces the taps and replicates y to
    all partitions, and a DVE copy appends into the ring buffer.
"""
import os
import numpy as np

# ---- static problem shapes (from the nn_DiffKS spec; value-independent) ----
T = 44100
NFRAMES = 100
NCO = 6            # coeff count
NACT = 7           # active taps
BURST = 2048
EXC_ORD = 5

B = 96             # block size (<= min delay 100 + 1)
PER = 6            # ring rebase period in blocks
RING_Y = 1280      # y ring [0, 1280); write pos starts at 704
RING_X0 = 1280     # x ring start
RING_X = 672       # x ring length
NELEMS = RING_Y + RING_X   # 1952 = ap_gather num_elems
PMAJ_COLS = 345    # t = 345*p + col layout (128*345 = 44160)
T_PAD = 128 * PMAJ_COLS    # 44160
NBLK = T_PAD // B          # 460
NEUMANN = 12

_CACHE = {}


def _build_constants():
    """Shape-only constant tables (no dependence on input values)."""
    c = {}
    t_pm = (np.arange(128)[:, None] * PMAJ_COLS + np.arange(PMAJ_COLS)[None, :])
    pos = np.linspace(0.0, NFRAMES - 1.0, T, dtype=np.float64)
    pos = np.concatenate([pos, np.full(T_PAD - T, pos[-1])])
    i0 = np.clip(np.floor(pos).astype(np.int32), 0, NFRAMES - 2)
    w = (pos - i0).astype(np.float32)
    i0_pm = i0[t_pm]
    w_pm = w[t_pm]
    fbase = i0_pm[:, 0]
    c["m0"] = (i0_pm == fbase[:, None]).astype(np.float32)
    c["m1"] = (i0_pm == fbase[:, None] + 1).astype(np.float32)
    c["w"] = w_pm.astype(np.float32)
    c["ohd"] = np.zeros((NFRAMES, 128), np.float32)
    c["ohd"][fbase, np.arange(128)] = 1.0
    c["i96"] = (t_pm % B).astype(np.float32)
    te = (np.arange(128)[:, None] * 16 + np.arange(16)[None, :])
    pe = np.linspace(0.0, NFRAMES - 1.0, BURST, dtype=np.float64)
    i0e = np.clip(np.floor(pe).astype(np.int32), 0, NFRAMES - 2)
    we = (pe - i0e).astype(np.float32)
    i0e_pm = i0e[te]
    we_pm = we[te]
    febase = i0e_pm[:, 0]
    c["m0e"] = (i0e_pm == febase[:, None]).astype(np.float32)
    c["m1e"] = (i0e_pm == febase[:, None] + 1).astype(np.float32)
    c["we"] = we_pm.astype(np.float32)
    c["ohe"] = np.zeros((NFRAMES, 128), np.float32)
    c["ohe"][febase, np.arange(128)] = 1.0
    ones = np.zeros((128, 128), np.float32)
    for j in range(8):
        ones[16 * j, :] = 1.0
    c["ones"] = ones
    addc = np.zeros((128, 1), np.float32)
    for j in range(7):
        addc[16 * j:16 * j + 16] = j + 704
    addc[112:] = RING_X0
    c["addc"] = addc
    c["i96w"] = (np.arange(16)[:, None] + 16 * np.arange(6)[None, :]).astype(np.float32)
    return c


def _build_program(nblk_limit=None):
    import concourse.bass as bass
    import concourse.tile as tile
    from concourse import bacc, mybir
    from contextlib import ExitStack

    f32 = mybir.dt.float32
    i16 = mybir.dt.int16
    i32 = mybir.dt.int32

    nblk = NBLK if nblk_limit is None else min(nblk_limit, NBLK)

    nc = bacc.Bacc("TRN2", target_bir_lowering=False, debug=False,
                   enable_asserts=False, num_devices=8)

    # ---------------- DRAM I/O ----------------
    d_delay = nc.dram_tensor("delay_frames", [NFRAMES], f32, kind="ExternalInput").ap()
    d_exc = nc.dram_tensor("excitation", [BURST], f32, kind="ExternalInput").ap()
    d_rcf = nc.dram_tensor("raw_coeff", [NFRAMES, NCO], f32, kind="ExternalInput").ap()
    d_gain = nc.dram_tensor("raw_gain", [1], f32, kind="ExternalInput").ap()
    d_ecf = nc.dram_tensor("exc_coeff", [NFRAMES, EXC_ORD], f32, kind="ExternalInput").ap()
    d_m0 = nc.dram_tensor("c_m0", [128, PMAJ_COLS], f32, kind="ExternalInput").ap()
    d_m1 = nc.dram_tensor("c_m1", [128, PMAJ_COLS], f32, kind="ExternalInput").ap()
    d_w = nc.dram_tensor("c_w", [128, PMAJ_COLS], f32, kind="ExternalInput").ap()
    d_ohd = nc.dram_tensor("c_ohd", [NFRAMES, 128], f32, kind="ExternalInput").ap()
    d_i96 = nc.dram_tensor("c_i96", [128, PMAJ_COLS], f32, kind="ExternalInput").ap()
    d_m0e = nc.dram_tensor("c_m0e", [128, 16], f32, kind="ExternalInput").ap()
    d_m1e = nc.dram_tensor("c_m1e", [128, 16], f32, kind="ExternalInput").ap()
    d_we = nc.dram_tensor("c_we", [128, 16], f32, kind="ExternalInput").ap()
    d_ohe = nc.dram_tensor("c_ohe", [NFRAMES, 128], f32, kind="ExternalInput").ap()
    d_ones = nc.dram_tensor("c_ones", [128, 128], f32, kind="ExternalInput").ap()
    d_addc = nc.dram_tensor("c_addc", [128, 1], f32, kind="ExternalInput").ap()
    d_i96w = nc.dram_tensor("c_i96w", [16, 6], f32, kind="ExternalInput").ap()
    d_y = nc.dram_tensor("y", [T], f32, kind="ExternalOutput").ap()
    dbg_b = int(os.environ.get("DIFFKS_DBGB", "-1"))
    d_dbg = nc.dram_tensor("dbg", [3, 128, B], f32, kind="ExternalOutput").ap() if dbg_b >= 0 else None

    # DRAM scratch
    d_dpad = nc.dram_tensor("s_dpad", [NFRAMES + 2], f32, kind="Internal").ap()
    d_nrm = nc.dram_tensor("s_nrm", [NFRAMES + 2, NCO], f32, kind="Internal").ap()
    d_ecf2 = nc.dram_tensor("s_ecf2", [NFRAMES + 2, EXC_ORD], f32, kind="Internal").ap()
    d_ct = nc.dram_tensor("s_ct", [NACT, T_PAD], f32, kind="Internal").ap()
    d_cstg = nc.dram_tensor("s_cstg", [NBLK, 8, B], f32, kind="Internal").ap()
    d_slin = nc.dram_tensor("s_slin", [T_PAD], f32, kind="Internal").ap()
    d_xlin = nc.dram_tensor("s_xlin", [T_PAD + 672], f32, kind="Internal").ap()
    d_ydump = nc.dram_tensor("s_ydump", [T_PAD], f32, kind="Internal").ap()
    d_gsc = nc.dram_tensor("s_gsc", [1], f32, kind="Internal").ap()

    with tile.TileContext(nc) as tc:
        with ExitStack() as ctx:
            sing = ctx.enter_context(tc.tile_pool(name="sing", bufs=1))
            work = ctx.enter_context(tc.tile_pool(name="work", bufs=2))
            cpool = ctx.enter_context(tc.tile_pool(name="cpool", bufs=6))
            ipool = ctx.enter_context(tc.tile_pool(name="ipool", bufs=6))
            psum = ctx.enter_context(tc.tile_pool(name="psum", bufs=2, space="PSUM"))

            AL = mybir.AluOpType
            ACT = mybir.ActivationFunctionType

            # ======== constants ========
            m0 = sing.tile([128, PMAJ_COLS], f32); nc.sync.dma_start(m0[:], d_m0)
            m1 = sing.tile([128, PMAJ_COLS], f32); nc.sync.dma_start(m1[:], d_m1)
            wt = sing.tile([128, PMAJ_COLS], f32); nc.sync.dma_start(wt[:], d_w)
            i96 = sing.tile([128, PMAJ_COLS], f32); nc.sync.dma_start(i96[:], d_i96)
            ohd = sing.tile([NFRAMES, 128], f32); nc.sync.dma_start(ohd[:], d_ohd)
            ohe = sing.tile([NFRAMES, 128], f32); nc.sync.dma_start(ohe[:], d_ohe)
            m0e = sing.tile([128, 16], f32); nc.sync.dma_start(m0e[:], d_m0e)
            m1e = sing.tile([128, 16], f32); nc.sync.dma_start(m1e[:], d_m1e)
            we = sing.tile([128, 16], f32); nc.sync.dma_start(we[:], d_we)
            ones_t = sing.tile([128, 128], f32); nc.sync.dma_start(ones_t[:], d_ones)
            addc = sing.tile([128, 1], f32); nc.sync.dma_start(addc[:], d_addc)

            # ======== frame-level prep ========
            rcf = sing.tile([NFRAMES, NCO], f32)
            nc.sync.dma_start(rcf[:], d_rcf)
            sb = sing.tile([NFRAMES, NCO], f32)
            nc.scalar.activation(sb[:], rcf[:], ACT.Sigmoid)
            ssum = sing.tile([NFRAMES, 1], f32)
            nc.vector.tensor_reduce(ssum[:], sb[:], op=AL.add, axis=mybir.AxisListType.X)
            rsum = sing.tile([NFRAMES, 1], f32)
            nc.vector.reciprocal(rsum[:], ssum[:])
            gain_t = sing.tile([1, 1], f32)
            nc.sync.dma_start(gain_t[:], d_gain[None, :])
            sgain = sing.tile([1, 1], f32)
            nc.scalar.activation(sgain[:], gain_t[:], ACT.Sigmoid)
            nc.vector.tensor_scalar(sgain[:], sgain[:], 0.1, 0.9, op0=AL.mult, op1=AL.add)
            nc.sync.dma_start(d_gsc[None, :], sgain[:])
            gbc = sing.tile([NFRAMES, 1], f32)
            nc.sync.dma_start(gbc[:], d_gsc[None, :].to_broadcast((NFRAMES, 1)))
            nrm = sing.tile([NFRAMES, NCO], f32)
            nc.vector.tensor_scalar(nrm[:], sb[:], rsum[:], None, op0=AL.mult)
            nc.vector.tensor_scalar(nrm[:], nrm[:], gbc[:], None, op0=AL.mult)

            # zero-padded DRAM copies (avoid OOB reads in the window staging)
            zpad6 = sing.tile([2, NCO], f32)
            nc.vector.memset(zpad6[:], 0.0)
            nc.sync.dma_start(d_nrm[0:NFRAMES, :], nrm[:])
            nc.sync.dma_start(d_nrm[NFRAMES:NFRAMES + 2, :], zpad6[:])
            ecf = sing.tile([NFRAMES, EXC_ORD], f32)
            nc.sync.dma_start(ecf[:], d_ecf)
            zpad5 = sing.tile([2, EXC_ORD], f32)
            nc.vector.memset(zpad5[:], 0.0)
            nc.sync.dma_start(d_ecf2[0:NFRAMES, :], ecf[:])
            nc.sync.dma_start(d_ecf2[NFRAMES:NFRAMES + 2, :], zpad5[:])
            dfr = sing.tile([1, NFRAMES], f32)
            nc.sync.dma_start(dfr[:], d_delay[None, :])
            zpad2 = sing.tile([1, 2], f32)
            nc.vector.memset(zpad2[:], 0.0)
            nc.sync.dma_start(d_dpad[0:NFRAMES][None, :], dfr[:])
            nc.sync.dma_start(d_dpad[NFRAMES:NFRAMES + 2][None, :], zpad2[:])

            # windowed stages (f, c) with c = 0..2
            dstage = sing.tile([NFRAMES, 3], f32)
            nc.sync.dma_start(dstage[:], bass.AP(
                tensor=d_dpad.tensor, offset=0, ap=[[1, NFRAMES], [1, 3]]))
            cfstage = sing.tile([NFRAMES, 3, NCO], f32)
            nc.sync.dma_start(cfstage[:], bass.AP(
                tensor=d_nrm.tensor, offset=0,
                ap=[[NCO, NFRAMES], [NCO, 3], [1, NCO]]))
            ecstage = sing.tile([NFRAMES, 3, EXC_ORD], f32)
            nc.sync.dma_start(ecstage[:], bass.AP(
                tensor=d_ecf2.tensor, offset=0,
                ap=[[EXC_ORD, NFRAMES], [EXC_ORD, 3], [1, EXC_ORD]]))

            # one-hot gathers to per-partition frame values
            ps_d = psum.tile([128, 3], f32, tag="psd")
            nc.tensor.matmul(ps_d[:], lhsT=ohd[:], rhs=dstage[:], start=True, stop=True)
            dfp = sing.tile([128, 3], f32)
            nc.vector.tensor_copy(dfp[:], ps_d[:])
            ps_c = psum.tile([128, 3 * NCO], f32, tag="psd")
            nc.tensor.matmul(ps_c[:], lhsT=ohd[:],
                             rhs=cfstage[:].rearrange("p a b -> p (a b)"),
                             start=True, stop=True)
            cfp = sing.tile([128, 3, NCO], f32)
            nc.vector.tensor_copy(cfp[:].rearrange("p a b -> p (a b)"), ps_c[:])
            ps_e = psum.tile([128, 3 * EXC_ORD], f32, tag="psd")
            nc.tensor.matmul(ps_e[:], lhsT=ohe[:],
                             rhs=ecstage[:].rearrange("p a b -> p (a b)"),
                             start=True, stop=True)
            efp = sing.tile([128, 3, EXC_ORD], f32)
            nc.vector.tensor_copy(efp[:].rearrange("p a b -> p (a b)"), ps_e[:])

            # ======== control signals in pmaj layout ========
            def lerp_tile(out, lo_s, mid_s, hi_s, msk0, msk1, wfac, cols):
                tlo = work.tile([128, cols], f32, tag="lerp_a")
                t2 = work.tile([128, cols], f32, tag="lerp_b")
                nc.vector.tensor_scalar(tlo[:], msk0, lo_s, None, op0=AL.mult)
                nc.vector.tensor_scalar(t2[:], msk1, mid_s, None, op0=AL.mult)
                nc.vector.tensor_add(tlo[:], tlo[:], t2[:])
                thi = work.tile([128, cols], f32, tag="lerp_c")
                nc.vector.tensor_scalar(thi[:], msk0, mid_s, None, op0=AL.mult)
                nc.vector.tensor_scalar(t2[:], msk1, hi_s, None, op0=AL.mult)
                nc.vector.tensor_add(thi[:], thi[:], t2[:])
                nc.vector.tensor_sub(thi[:], thi[:], tlo[:])
                nc.vector.tensor_mul(thi[:], thi[:], wfac)
                nc.vector.tensor_add(out, tlo[:], thi[:])

            delay = sing.tile([128, PMAJ_COLS], f32)
            lerp_tile(delay[:], dfp[:, 0:1], dfp[:, 1:2], dfp[:, 2:3],
                      m0[:], m1[:], wt[:], PMAJ_COLS)
            zi = sing.tile([128, PMAJ_COLS], i32)
            nc.vector.tensor_copy(zi[:], delay[:])
            zf = sing.tile([128, PMAJ_COLS], f32)
            nc.vector.tensor_copy(zf[:], zi[:])
            corr = work.tile([128, PMAJ_COLS], f32, tag="corr")
            nc.vector.tensor_tensor(corr[:], zf[:], delay[:], AL.is_gt)
            nc.vector.tensor_sub(zf[:], zf[:], corr[:])
            alfa = sing.tile([128, PMAJ_COLS], f32)
            nc.vector.tensor_sub(alfa[:], delay[:], zf[:])

            bcols = sing.tile([128, NCO, PMAJ_COLS], f32)
            for k in range(NCO):
                lerp_tile(bcols[:, k, :], cfp[:, 0, k:k + 1], cfp[:, 1, k:k + 1],
                          cfp[:, 2, k:k + 1], m0[:], m1[:], wt[:], PMAJ_COLS)
            am1 = sing.tile([128, PMAJ_COLS], f32)
            nc.vector.tensor_scalar(am1[:], alfa[:], -1.0, 1.0, op0=AL.mult, op1=AL.add)
            ctile = sing.tile([128, NACT, PMAJ_COLS], f32)
            nc.vector.tensor_mul(ctile[:, 0, :], am1[:], bcols[:, 0, :])
            tmpc = work.tile([128, PMAJ_COLS], f32, tag="tmpc")
            for j in range(1, NCO):
                nc.vector.tensor_mul(ctile[:, j, :], alfa[:], bcols[:, j - 1, :])
                nc.vector.tensor_mul(tmpc[:], am1[:], bcols[:, j, :])
                nc.vector.tensor_add(ctile[:, j, :], ctile[:, j, :], tmpc[:])
            nc.vector.tensor_mul(ctile[:, NCO, :], alfa[:], bcols[:, NCO - 1, :])

            srel = sing.tile([128, PMAJ_COLS], f32)
            nc.vector.tensor_sub(srel[:], i96[:], zf[:])
            nc.vector.tensor_scalar(srel[:], srel[:], -7.0, None, op0=AL.add)

            # ======== stage c and s via DRAM ========
            nc.sync.dma_start(
                d_ct.rearrange("j (p c) -> p j c", p=128), ctile[:])
            for j in range(NACT):
                # gather core j holds y[t-z-7+j], i.e. tap (6-j): reverse order
                nc.sync.dma_start(
                    d_cstg[:, j, :],
                    d_ct[NACT - 1 - j:NACT - j, :].rearrange("a (b i) -> (a b) i", i=B))
            # ones row (j=7) of cstg, in 128-block chunks
            ones_blk = sing.tile([128, B], f32)
            nc.vector.memset(ones_blk[:], 1.0)
            done = 0
            while done < NBLK:
                n = min(128, NBLK - done)
                nc.sync.dma_start(d_cstg[done:done + n, 7, :], ones_blk[0:n, :])
                done += n

            nc.sync.dma_start(
                d_slin.rearrange("(p c) -> p c", p=128), srel[:])
            s_rep = sing.tile([128, 6 * NBLK], f32)
            srep_src = d_slin.rearrange("(b q m) -> m b q", b=NBLK, q=6, m=16)
            for j in range(NACT):
                nc.sync.dma_start(
                    s_rep[16 * j:16 * j + 16, :].rearrange("p (b q) -> p b q", q=6),
                    srep_src)
            nc.sync.dma_start(
                s_rep[112:128, :].rearrange("p (b q) -> p b q", q=6),
                d_i96w[:, None, :].to_broadcast((16, NBLK, 6)))

            # ======== excitation (Neumann) ========
            acoef = sing.tile([128, EXC_ORD, 16], f32)
            for k in range(EXC_ORD):
                lerp_tile(acoef[:, k, :], efp[:, 0, k:k + 1], efp[:, 1, k:k + 1],
                          efp[:, 2, k:k + 1], m0e[:], m1e[:], we[:], 16)
            x0 = sing.tile([128, 16], f32)
            nc.sync.dma_start(x0[:], d_exc.rearrange("(p c) -> p c", p=128))
            ycat = sing.tile([128, 32], f32)
            nc.vector.memset(ycat[:, 0:16], 0.0)
            nc.vector.tensor_copy(ycat[:, 16:32], x0[:])
            acc = sing.tile([128, 16], f32)
            prod = work.tile([128, 16], f32, tag="nprod")
            for it in range(NEUMANN):
                nc.vector.memset(ycat[0:1, 0:16], 0.0)
                nc.sync.dma_start(ycat[1:128, 0:16], ycat[0:127, 16:32])
                nc.vector.tensor_mul(acc[:], acoef[:, 0, :], ycat[:, 15:31])
                for k in range(1, EXC_ORD):
                    nc.vector.tensor_mul(prod[:], acoef[:, k, :], ycat[:, 15 - k:31 - k])
                    nc.vector.tensor_add(acc[:], acc[:], prod[:])
                nc.vector.tensor_sub(acc[:], x0[:], acc[:])
                nc.vector.tensor_copy(ycat[:, 16:32], acc[:])
            zb = work.tile([128, PMAJ_COLS], f32, tag="zb")
            nc.vector.memset(zb[:], 0.0)
            nc.sync.dma_start(
                d_xlin[0:T_PAD].rearrange("(p c) -> p c", p=128), zb[:])
            ztail = sing.tile([1, 672], f32)
            nc.vector.memset(ztail[:], 0.0)
            nc.sync.dma_start(d_xlin[T_PAD:T_PAD + 672][None, :], ztail[:])
            nc.sync.dma_start(
                d_xlin[0:BURST].rearrange("(p c) -> p c", p=128), ycat[:, 16:32])

            # ======== main chain ========
            hx = sing.tile([128, NELEMS], f32)
            nc.vector.memset(hx[:], 0.0)
            nc.sync.dma_start(hx[112:113, RING_X0:RING_X0 + RING_X],
                              d_xlin[0:RING_X][None, :])

            nperiods = nblk // PER
            rem = nblk - nperiods * PER

            # zero the rotating coefficient buffers once (the per-block DMA
            # only writes partition rows {16j}; the rest must stay 0)
            for _ in range(6):
                zt = cpool.tile([128, B], f32, tag="ct")
                nc.vector.memset(zt[:], 0.0)

            with nc.named_scope("mainchain"):
                bidx = 0
                for per in range(nperiods + 1):
                    blocks = PER if per < nperiods else rem
                    if blocks == 0:
                        break
                    for r in range(blocks):
                        b = bidx
                        idx_f = ipool.tile([128, 6], f32, tag="idxf")
                        nc.vector.tensor_scalar(idx_f[:], s_rep[:, 6 * b:6 * b + 6],
                                                addc[:], float(96 * r),
                                                op0=AL.add, op1=AL.add)
                        idx_t = ipool.tile([128, 6], i16, tag="idx")
                        nc.vector.tensor_copy(idx_t[:], idx_f[:])
                        c_t = cpool.tile([128, B], f32, tag="ct")
                        c_dst = c_t[:].rearrange("(j m) f -> j m f", m=16)[:, 0, :]
                        nc.sync.dma_start(c_dst, d_cstg[b, :, :])
                        G = work.tile([128, B], f32, tag="G")
                        nc.gpsimd.ap_gather(G[:], hx[:], idx_t[:],
                                            channels=128, num_elems=NELEMS, d=1,
                                            num_idxs=B)
                        mulr = work.tile([128, B], f32, tag="mulr")
                        nc.vector.tensor_mul(mulr[:], G[:], c_t[:])
                        if dbg_b == b:
                            nc.sync.dma_start(d_dbg[0], G[:])
                            nc.sync.dma_start(d_dbg[1], c_t[:])
                            idxf_dbg = ipool.tile([128, 6], f32, tag="idxf")
                            nc.vector.tensor_copy(idxf_dbg[:], idx_t[:])
                            nc.sync.dma_start(d_dbg[2, :, 0:6], idxf_dbg[:])
                        yps = psum.tile([128, B], f32, tag="yps")
                        nc.tensor.matmul(yps[:], lhsT=ones_t[:], rhs=mulr[:],
                                         start=True, stop=True)
                        wp = 704 + 96 * r
                        nc.vector.tensor_copy(hx[:, wp:wp + B], yps[:])
                        bidx += 1
                    wend = 704 + 96 * blocks
                    t0 = (bidx - blocks) * B
                    nc.sync.dma_start(d_ydump[t0:t0 + 96 * blocks][None, :],
                                      hx[0:1, 704:wend])
                    if per < nperiods:
                        nc.vector.tensor_copy(hx[:, 0:704], hx[:, wend - 704:wend])
                        nc.sync.dma_start(hx[112:113, RING_X0:RING_X0 + RING_X],
                                          d_xlin[bidx * B:bidx * B + RING_X][None, :])

            # ======== output ========
            nc.sync.dma_start(d_y[None, :], d_ydump[0:T][None, :])

    nc.compile()
    return nc


def _run(nc, in_map, n_cores=8, trace=False):
    import concourse.bass_utils as bass_utils
    return bass_utils.run_bass_kernel_spmd(
        nc, [dict(in_map) for _ in range(n_cores)],
        core_ids=list(range(n_cores)), trace=trace)


def _make_in_map(delay_len_frames, excitation, raw_coeff_frames, raw_gain,
                 exc_coefficients):
    c = _CACHE["consts"]
    return {
        "delay_frames": np.ascontiguousarray(delay_len_frames, np.float32),
        "excitation": np.ascontiguousarray(excitation, np.float32),
        "raw_coeff": np.ascontiguousarray(raw_coeff_frames, np.float32),
        "raw_gain": np.asarray(raw_gain, np.float32).reshape(1),
        "exc_coeff": np.ascontiguousarray(exc_coefficients, np.float32),
        "c_m0": c["m0"], "c_m1": c["m1"], "c_w": c["w"], "c_ohd": c["ohd"],
        "c_i96": c["i96"], "c_m0e": c["m0e"], "c_m1e": c["m1e"], "c_we": c["we"],
        "c_ohe": c["ohe"], "c_ones": c["ones"], "c_addc": c["addc"],
        "c_i96w": c["i96w"],
    }


def kernel(delay_len_frames, excitation, raw_coeff_frames, raw_gain,
           exc_coefficients, n_samples):
    assert int(n_samples) == T
    if "nc" not in _CACHE:
        _CACHE["consts"] = _build_constants()
        nblk_limit = os.environ.get("DIFFKS_NBLK")
        _CACHE["nc"] = _build_program(int(nblk_limit) if nblk_limit else None)
    res = _run(_CACHE["nc"],
               _make_in_map(delay_len_frames, excitation, raw_coeff_frames,
                            raw_gain, exc_coefficients))
    return res.results[0]["y"].astype(np.float32)
